# revision 32
# baseline (speedup 1.0000x reference)
"""Differentiable AAC forward pass on 8 Trainium2 NeuronCores.

Data-parallel over batch (8 batches -> 8 cores). Per core:
 - MDCT as an unfolded 2048-contraction matmul in f32r against the
   reference's own fp32-computed (window * cosine) matrix; the input
   x is transposed once per 128-frame block on the PE (consecutive
   frames overlap, so block b's transpose also provides the "sliver"
   column that block b-1 needs for its second half).
 - |c|^0.75 via Ln/Exp on the ACT engine; per-frame integer gain via a
   6-iteration binary search over [0, 30] (trajectory-identical to the
   reference's 8-iteration [0,120] search whenever frame bits at gains
   60 and 30 are <= TARGET, which holds for any sane audio), counting
   exact exponent bits of bf16(ax*inv + 0.5) on the DVE.
 - quantize/dequantize on the ACT engine (round via +/-magic, then
   exp((4/3)ln(q) + g*ln2/4)), sign restored from the MDCT psum.
 - IMDCT in bf16 with the overlap-add fused into PSUM accumulation.

The per-block stages are software-pipelined 5 deep, and each outer step
interleaves the serial ACT<->DVE ping-pong of TWO independent gain
searches (blocks at different pipeline depths, 3 iterations each) with
the MDCT chunks, so the in-order engine queues always have independent
work during the searches' cross-engine waits.
"""

import numpy as np

import concourse.bass as bass
import concourse.bacc as bacc
import concourse.mybir as mybir
import concourse.tile as tile
from concourse.bass_utils import run_bass_kernel_spmd

M = 1024
N2 = 2048
NCORES = 8
MAGIC = 12582912.0          # 1.5 * 2^23, RNE-to-integer magic for |v| < 2^22
LN2 = 0.6931471805599453
TARGET_BITS = 128000 * 1024 / 48000.0   # 2730.666... bits per frame
THRESH_I = 258730           # too_big <=> sum(E) > floor(TARGET + 125*2048)
GAIN_HI = 30.0              # narrowed search range (see module docstring)
SEARCH_ITERS = 6            # covers [0, 30] exactly as ref iters 3..8
INV0 = float(np.exp2(np.float64(-0.1875 * 15.0)))  # iter-0 mid is always 15
MDCT_F32 = False            # fallback: fp32 MDCT (4x slower PE, ~10x less err)


def _f32(x):
    return np.float32(x)


def _bf16(x):
    import ml_dtypes
    return np.ascontiguousarray(x.astype(ml_dtypes.bfloat16))


def host_constants():
    """Basis matrices matching the reference's fp32 computation bit-for-bit
    (jnp on CPU when available, else a numpy replica that matches to 1 ulp)."""
    try:
        import jax
        import jax.numpy as jnp
        cpu = jax.devices("cpu")[0]
        with jax.default_device(cpu):
            n = jnp.arange(N2, dtype=jnp.float32)
            w = np.asarray(jnp.sin(jnp.pi / N2 * (n + 0.5)))
            nn = jnp.arange(N2, dtype=jnp.float32)[:, None]
            kk = jnp.arange(M, dtype=jnp.float32)[None, :]
            Cm = np.asarray(jnp.cos(jnp.pi / M * (nn + 0.5 + M / 2) * (kk + 0.5)))
    except Exception:
        n = np.arange(N2, dtype=np.float32)
        w = np.sin((_f32(np.pi / N2) * (n + _f32(0.5))).astype(np.float32))
        w = w.astype(np.float32)
        nn = np.arange(N2, dtype=np.float32)[:, None]
        kk = np.arange(M, dtype=np.float32)[None, :]
        arg = (_f32(np.pi / M) * (nn + _f32(0.5) + _f32(M / 2))).astype(np.float32)
        arg = (arg * (kk + _f32(0.5))).astype(np.float32)
        Cm = np.cos(arg).astype(np.float32)

    Cw = (w[:, None] * Cm).astype(np.float32)            # (N2, M) analysis
    Cw2 = ((_f32(2.0 / M) * w)[:, None] * Cm).astype(np.float32)  # synthesis
    R1 = np.ascontiguousarray(Cw2[:M].T)                 # (M k, M r) A-half
    R2 = np.ascontiguousarray(Cw2[M:].T)                 # (M k, M r) B-half

    def lay(a):  # (1024, 1024) -> (128, 8, 1024) [p, t, c] = a[t*128+p, c]
        return np.ascontiguousarray(
            a.astype(np.float32).reshape(8, 128, M).transpose(1, 0, 2))

    consts = {
        "cwa": lay(Cw[:M]),
        "cwb": lay(Cw[M:]),
        "r1": _bf16(lay(R1)),
        "r2": _bf16(lay(R2)),
        "ident": np.eye(128, dtype=np.float32),
    }
    return consts


def build_nc(nb, nrows, ncores=NCORES, reps=1):
    """Build the per-core Bass kernel.

    nb:    number of 128-frame blocks (frames F = nb*128)
    nrows: rows of the padded input X (= F + 1)
    """
    F = nb * 128
    out_len = F * M

    nc = bacc.Bacc("TRN2", target_bir_lowering=False, debug=False,
                   num_devices=ncores)
    f32 = mybir.dt.float32
    f32r = mybir.dt.float32r
    bf16 = mybir.dt.bfloat16
    i32 = mybir.dt.int32
    u16 = mybir.dt.uint16
    Alu = mybir.AluOpType
    Act = mybir.ActivationFunctionType

    mdt = f32 if MDCT_F32 else f32r

    x_d = nc.dram_tensor("x", [2, nrows, M], mdt, kind="ExternalInput")
    cwa_d = nc.dram_tensor("cwa", [128, 8, M], mdt, kind="ExternalInput")
    cwb_d = nc.dram_tensor("cwb", [128, 8, M], mdt, kind="ExternalInput")
    r1_d = nc.dram_tensor("r1", [128, 8, M], bf16, kind="ExternalInput")
    r2_d = nc.dram_tensor("r2", [128, 8, M], bf16, kind="ExternalInput")
    id_d = nc.dram_tensor("ident", [128, 128], f32, kind="ExternalInput")
    out_d = nc.dram_tensor("out", [2, out_len], f32, kind="ExternalOutput")

    def x_slice(c, r0, nr):
        return bass.AP(tensor=x_d, offset=(c * nrows + r0) * M,
                       ap=[[M, nr], [1, M]])

    def out_slice(c, blk0, npart, r0, nr):
        return bass.AP(tensor=out_d, offset=c * out_len + blk0 * M + r0,
                       ap=[[M, npart], [1, nr]])

    with tile.TileContext(nc) as tc:
        import contextlib
        ctx = contextlib.ExitStack()
        with ctx:
            consts = ctx.enter_context(tc.tile_pool(name="consts", bufs=1))
            xin = ctx.enter_context(tc.tile_pool(name="xin", bufs=1))
            xtp = ctx.enter_context(tc.tile_pool(name="xtp", bufs=2))
            axp = ctx.enter_context(tc.tile_pool(name="axp", bufs=4))
            sgp = ctx.enter_context(tc.tile_pool(name="sgp", bufs=4))
            zp = ctx.enter_context(tc.tile_pool(name="zp", bufs=2))
            qp = ctx.enter_context(tc.tile_pool(name="qp", bufs=2))
            dqp = ctx.enter_context(tc.tile_pool(name="dqp", bufs=1))
            dqtp = ctx.enter_context(tc.tile_pool(name="dqtp", bufs=3))
            outp = ctx.enter_context(tc.tile_pool(name="outp", bufs=2))
            stat = ctx.enter_context(tc.tile_pool(name="stat", bufs=2))
            lhp = ctx.enter_context(tc.tile_pool(name="lhp", bufs=2))
            psT = ctx.enter_context(tc.tile_pool(name="psT", bufs=2, space="PSUM"))
            psM = ctx.enter_context(tc.tile_pool(name="psM", bufs=2, space="PSUM"))
            psQ = ctx.enter_context(tc.tile_pool(name="psQ", bufs=2, space="PSUM"))
            psI = ctx.enter_context(tc.tile_pool(name="psI", bufs=2, space="PSUM"))

            id_sb = consts.tile([128, 128], f32)
            nc.sync.dma_start(out=id_sb, in_=id_d[:, :])
            idm_sb = consts.tile([128, 128], mdt)
            nc.vector.tensor_copy(out=idm_sb, in_=id_sb)
            # first x blocks are loaded (emitted in the pre-loop
            # load_transpose calls below) before the big basis matrices so
            # the transposes can start immediately; split the basis DMAs by
            # k-half so the first MDCT matmuls only wait for half
            cwa_sb = consts.tile([128, 8, M], mdt)
            cwb_sb = consts.tile([128, 8, M], mdt)
            r1_sb = consts.tile([128, 8, M], bf16)
            r2_sb = consts.tile([128, 8, M], bf16)

            def load_consts():
                nc.sync.dma_start(out=cwa_sb[:, :, 0:512],
                                  in_=cwa_d[:, :, 0:512])
                nc.sync.dma_start(out=cwb_sb[:, :, 0:512],
                                  in_=cwb_d[:, :, 0:512])
                nc.sync.dma_start(out=cwa_sb[:, :, 512:M],
                                  in_=cwa_d[:, :, 512:M])
                nc.sync.dma_start(out=cwb_sb[:, :, 512:M],
                                  in_=cwb_d[:, :, 512:M])
                nc.sync.dma_start(out=r1_sb, in_=r1_d[:, :, :])
                nc.sync.dma_start(out=r2_sb, in_=r2_d[:, :, :])
            idb_sb = consts.tile([128, 128], bf16)
            nc.vector.tensor_copy(out=idb_sb, in_=id_sb)
            eps35 = consts.tile([128, 1], f32)
            nc.vector.memset(eps35, 1e-35)
            zero8 = consts.tile([128, 8], f32)
            nc.vector.memset(zero8, 0.0)

            # rings: xT [b%2][c]; dqT [b%3][c] -> (128, 8, 129) tiles
            xt_ring = [[None, None], [None, None]]
            dqt_ring = [[None, None], [None, None], [None, None]]
            ax_t = {}
            sgn_t = {}
            lnt_t = {}
            lo_t = {}
            hi_t = {}

            def load_transpose(b):
                """DMA x rows [b*128,+128) and transpose into xT ring; write
                sliver col 128 of ring (b-1)."""
                par = b % 2
                for c in range(2):
                    xc = xin.tile([128, M], mdt, name=f"xc_{b}_{c}", tag="xin")
                    nc.sync.dma_start(out=xc, in_=x_slice(c, b * 128, 128))
                    buf = xtp.tile([128, 8, 129], mdt, name=f"xt_{b}_{c}",
                                   tag=f"xt{c}")
                    xt_ring[par][c] = buf
                    for g in range(2):
                        pst = psT.tile([128, 512], mdt, name=f"pst_{b}_{c}_{g}",
                                       tag="pst")
                        for j in range(4):
                            jt = g * 4 + j
                            nc.tensor.transpose(
                                pst[:, j * 128:(j + 1) * 128],
                                xc[:, jt * 128:(jt + 1) * 128], idm_sb)
                        nc.vector.tensor_copy(
                            out=buf[:, g * 4:(g + 1) * 4, 0:128], in_=pst)
                        if b > 0:
                            prev = xt_ring[1 - par][c]
                            slin = bass.AP(tensor=pst.tensor, offset=pst.offset,
                                           ap=[pst.ap[0], [128, 4]])
                            slout = bass.AP(
                                tensor=prev.tensor,
                                offset=prev.offset + (g * 4) * 129 + 128,
                                ap=[prev.ap[0], [129, 4]])
                            nc.vector.tensor_copy(out=slout, in_=slin)

            def mdct_chunks(b):
                """Emission thunks for block b's MDCT + ax75 chain.

                Returns (mm_thunks, abs_sign_thunk, ln_exp_thunk): the psum
                drain is a cheap DVE copy into `co` right behind each chunk's
                matmuls so the PE never waits on the ACT engine; Abs/Sign run
                as single full-width ACT ops from SBUF."""
                par = b % 2
                ax = axp.tile([128, 2, M], f32, name=f"ax_{b}", tag="ax")
                sgn = sgp.tile([128, 2, M], bf16, name=f"sg_{b}", tag="sg")
                co = qp.tile([128, 2, M], f32, name=f"co_{b}", tag="qa",
                             bufs=1)
                ax_t[b] = ax
                sgn_t[b] = sgn

                cocp = []

                def mm(c, kc):
                    def go():
                        buf = xt_ring[par][c]
                        psm = psM.tile([128, 512], f32,
                                       name=f"psm_{b}_{c}_{kc}", tag="psm")
                        ks = slice(kc * 512, (kc + 1) * 512)
                        for jt in range(8):
                            nc.tensor.matmul(psm, buf[:, jt, 0:128],
                                             cwa_sb[:, jt, ks],
                                             start=(jt == 0), stop=False)
                        for jt in range(8):
                            nc.tensor.matmul(psm, buf[:, jt, 1:129],
                                             cwb_sb[:, jt, ks],
                                             start=False, stop=(jt == 7))

                        def cp():
                            nc.vector.tensor_copy(out=co[:, c, ks], in_=psm)
                        cocp.append(cp)
                    return go

                def abs_sign():
                    nc.scalar.activation(out=ax, in_=co, func=Act.Abs)
                    nc.scalar.activation(out=sgn, in_=co, func=Act.Sign)

                def ln_exp():
                    lnt = qp.tile([128, 2, M], f32, name=f"ln_{b}", tag="qa",
                                  bufs=1)
                    nc.scalar.activation(out=lnt, in_=ax, func=Act.Ln,
                                         bias=eps35)
                    nc.scalar.activation(out=ax, in_=lnt, func=Act.Exp,
                                         scale=0.75)

                return ([mm(0, 0), mm(0, 1), mm(1, 0), mm(1, 1)],
                        cocp, abs_sign, ln_exp)

            def search_iter_thunks(b, its):
                """Emission thunks for search iterations `its` of block b."""
                def one(it):
                    def go():
                        if it == 0:
                            lo = lhp.tile([128, 1], f32, name=f"lo_{b}",
                                          tag="lo")
                            hi = lhp.tile([128, 1], f32, name=f"hi_{b}",
                                          tag="hi")
                            lo_t[b] = lo
                            hi_t[b] = hi
                            nc.vector.memset(lo, 0.0)
                            nc.vector.memset(hi, GAIN_HI)
                            mid = None
                            inv = None
                        else:
                            lo, hi = lo_t[b], hi_t[b]
                            t = stat.tile([128, 1], f32, name=f"t_{b}_{it}",
                                          tag="s1")
                            nc.vector.tensor_add(out=t, in0=lo, in1=hi)
                            mid = stat.tile([128, 1], f32, name=f"m_{b}_{it}",
                                            tag="s2")
                            nc.vector.tensor_scalar(out=mid, in0=t, scalar1=0.5,
                                                    scalar2=-0.25, op0=Alu.mult,
                                                    op1=Alu.add)
                            nc.vector.tensor_scalar(out=mid, in0=mid,
                                                    scalar1=MAGIC, scalar2=MAGIC,
                                                    op0=Alu.add,
                                                    op1=Alu.subtract)
                            inv = stat.tile([128, 1], f32, name=f"i_{b}_{it}",
                                            tag="s3")
                            nc.scalar.activation(out=inv, in_=mid, func=Act.Exp,
                                                 scale=-0.1875 * LN2)
                        z = zp.tile([128, 2, M], bf16, name=f"z_{b}_{it}",
                                    tag="z")
                        if it >= 3:
                            # second-half searches: z on the DVE to keep the
                            # ACT queue clear of the ping-pong
                            nc.vector.tensor_scalar(out=z, in0=ax_t[b],
                                                    scalar1=inv, scalar2=0.5,
                                                    op0=Alu.mult, op1=Alu.add)
                        else:
                            nc.scalar.activation(out=z, in_=ax_t[b],
                                                 func=Act.Copy,
                                                 scale=(INV0 if it == 0
                                                        else inv), bias=0.5)
                        e = z.bitcast(u16)
                        es = stat.tile([128, 1], f32, name=f"es_{b}_{it}",
                                       tag="s4")
                        with nc.allow_low_precision(reason="exact exp sums"):
                            nc.vector.tensor_scalar(out=e, in0=e,
                                                    scalar1=7, scalar2=None,
                                                    op0=Alu.logical_shift_right)
                            # f32 accumulator is exact for sums < 2^24
                            nc.vector.tensor_scalar(out=e, in0=e,
                                                    scalar1=1, scalar2=0,
                                                    op0=Alu.mult, op1=Alu.add,
                                                    accum_out=es)
                        msk = stat.tile([128, 1], i32, name=f"k_{b}_{it}",
                                        tag="s5")
                        nc.vector.tensor_scalar(out=msk, in0=es,
                                                scalar1=THRESH_I + 0.5,
                                                scalar2=None, op0=Alu.is_gt)
                        mskn = stat.tile([128, 1], i32, name=f"kn_{b}_{it}",
                                         tag="s6")
                        with nc.allow_low_precision(reason="int mask flip"):
                            nc.vector.tensor_scalar(out=mskn, in0=msk,
                                                    scalar1=-1, scalar2=1,
                                                    op0=Alu.mult, op1=Alu.add)
                        lo, hi = lo_t[b], hi_t[b]
                        mp1 = stat.tile([128, 1], f32, name=f"p_{b}_{it}",
                                        tag="s7")
                        if it == 0:
                            nc.vector.memset(mp1, 16.0)
                            mid0 = stat.tile([128, 1], f32, name=f"q_{b}_{it}",
                                             tag="s8")
                            nc.vector.memset(mid0, 15.0)
                            nc.vector.copy_predicated(out=lo, mask=msk,
                                                      data=mp1)
                            nc.vector.copy_predicated(out=hi, mask=mskn,
                                                      data=mid0)
                        else:
                            nc.vector.tensor_scalar(out=mp1, in0=mid,
                                                    scalar1=1.0, scalar2=None,
                                                    op0=Alu.add)
                            nc.vector.copy_predicated(out=lo, mask=msk,
                                                      data=mp1)
                            nc.vector.copy_predicated(out=hi, mask=mskn,
                                                      data=mid)
                    return go
                return [one(it) for it in its]

            def quant_block(b):
                """q = round(ax * 2^{-3g/16}); dq = sgn * q^{4/3} * 2^{g/4}.
                The f32 chain runs in place in one scratch tile on the ACT
                engine (elementwise, so in-place is safe)."""
                gains, ax, sgn = hi_t[b], ax_t[b], sgn_t[b]
                inv2 = stat.tile([128, 1], f32, name=f"v2_{b}", tag="s1")
                nc.scalar.activation(out=inv2, in_=gains, func=Act.Exp,
                                     scale=-0.1875 * LN2)
                lnscl = stat.tile([128, 1], f32, name=f"lsc_{b}", tag="s2")
                nc.vector.tensor_scalar(out=lnscl, in0=gains, scalar1=LN2 / 4.0,
                                        scalar2=None, op0=Alu.mult)
                qs = qp.tile([128, 2, M], f32, name=f"qs_{b}", tag="qa", bufs=1)
                nc.scalar.activation(out=qs, in_=ax, func=Act.Copy,
                                     scale=inv2, bias=MAGIC)
                nc.scalar.activation(out=qs, in_=qs, func=Act.Copy,
                                     bias=-MAGIC)
                nc.scalar.activation(out=qs, in_=qs, func=Act.Ln, bias=eps35)
                dqm = qp.tile([128, 2, M], bf16, name=f"dm_{b}", tag="qc", bufs=1)
                nc.scalar.activation(out=dqm, in_=qs, func=Act.Exp,
                                     scale=4.0 / 3.0, bias=lnscl)
                dq = dqp.tile([128, 2, M], bf16, name=f"dq_{b}", tag="dq")
                nc.vector.tensor_tensor(out=dq, in0=dqm, in1=sgn, op=Alu.mult)
                return dq

            def dqt_block(b, dq):
                """PE transposes now; returns 4 copy thunks (one per psum
                group) to interleave into the search rounds."""
                par = b % 3
                copies = []
                for c in range(2):
                    buf = dqtp.tile([128, 8, 129], bf16, name=f"dt_{b}_{c}",
                                    tag=f"dt{c}")
                    dqt_ring[par][c] = buf
                    for g in range(2):
                        psq = psQ.tile([128, 512], bf16, name=f"psq_{b}_{c}_{g}",
                                       tag="psq")
                        for j in range(4):
                            jt = g * 4 + j
                            nc.tensor.transpose(
                                psq[:, j * 128:(j + 1) * 128],
                                dq[:, c, jt * 128:(jt + 1) * 128], idb_sb)

                        def cp(c=c, g=g, psq=psq, buf=buf):
                            nc.vector.tensor_copy(
                                out=buf[:, g * 4:(g + 1) * 4, 0:128], in_=psq)
                            if b > 0:
                                prev = dqt_ring[(b - 1) % 3][c]
                                slin = bass.AP(tensor=psq.tensor,
                                               offset=psq.offset,
                                               ap=[psq.ap[0], [128, 4]])
                                slout = bass.AP(
                                    tensor=prev.tensor,
                                    offset=prev.offset + (g * 4) * 129 + 128,
                                    ap=[prev.ap[0], [129, 4]])
                                nc.vector.tensor_copy(out=slout, in_=slin)
                        copies.append(cp)
                return copies

            def imdct_block(bp):
                """PE matmuls now; returns 4 drain thunks (copy + DMA out)."""
                par = bp % 3
                copies = []
                for c in range(2):
                    buf = dqt_ring[par][c]
                    for rc in range(2):
                        psr = psI.tile([128, 512], f32, name=f"pr_{bp}_{c}_{rc}",
                                       tag="psr")
                        rs = slice(rc * 512, (rc + 1) * 512)
                        for kt in range(8):
                            nc.tensor.matmul(psr, buf[:, kt, 0:128],
                                             r2_sb[:, kt, rs],
                                             start=(kt == 0), stop=False)
                        for kt in range(8):
                            nc.tensor.matmul(psr, buf[:, kt, 1:129],
                                             r1_sb[:, kt, rs],
                                             start=False, stop=(kt == 7))

                        def cp(c=c, rc=rc, psr=psr):
                            ot = outp.tile([128, 512], f32,
                                           name=f"o_{bp}_{c}_{rc}", tag="ot")
                            nc.vector.tensor_copy(out=ot, in_=psr)
                            nc.sync.dma_start(
                                out=out_slice(c, bp * 128, 128, rc * 512, 512),
                                in_=ot)
                        copies.append(cp)
                return copies

            def memset_sliver(ring, b, mod=2):
                par = b % mod
                for c in range(2):
                    buf = ring[par][c]
                    sl = bass.AP(tensor=buf.tensor, offset=buf.offset + 128,
                                 ap=[buf.ap[0], [129, 8]])
                    nc.vector.tensor_copy(out=sl, in_=zero8)

            load_transpose(0)
            load_consts()
            for rep in range(reps):
              if rep > 0:
                  load_transpose(0)
              for b in range(nb + 6):
                if b + 1 == nb + 1:
                    pass
                if b + 1 == nb:
                    pass
                # quant for b-4 first: its search finished last iteration, so
                # the ACT ops run at queue front and dq is ready before the
                # PE reaches the dqT transposes below.
                dq = quant_block(b - 4) if 4 <= b <= nb + 3 else None
                if 1 <= b <= nb:
                    mms, cocp, abs_sign, ln_exp = mdct_chunks(b - 1)
                    for th in mms:
                        th()
                else:
                    cocp, abs_sign, ln_exp = [], None, None
                if b == nb + 4:
                    memset_sliver(dqt_ring, nb - 1, 3)
                im_cp = imdct_block(b - 6) if 6 <= b <= nb + 5 else []
                dqt_cp = dqt_block(b - 4, dq) if dq is not None else []
                sA = (search_iter_thunks(b - 2, range(0, 3))
                      if 2 <= b <= nb + 1 else [])
                sB = (search_iter_thunks(b - 3, range(3, SEARCH_ITERS))
                      if 3 <= b <= nb + 2 else [])
                rounds = []
                for i in range(max(len(sA), len(sB))):
                    if i < len(sA):
                        rounds.append(sA[i])
                    if i < len(sB):
                        rounds.append(sB[i])
                drains = cocp + im_cp + dqt_cp
                nr = max(len(rounds), 1)
                per = -(-len(drains) // nr)
                di = 0
                for i in range(nr):
                    if i < len(rounds):
                        rounds[i]()
                    for _ in range(per):
                        if di < len(drains):
                            drains[di]()
                            di += 1
                while di < len(drains):
                    drains[di]()
                    di += 1
                if abs_sign is not None:
                    abs_sign()
                if ln_exp is not None:
                    ln_exp()
                if b + 1 < nb:
                    load_transpose(b + 1)
                if b + 1 == nb:
                    memset_sliver(xt_ring, nb - 1)

    # All activation funcs used here (Exp/Ln/Copy/Abs/Sign) coexist in the
    # natural_log_exp_and_others table; by default the table chooser assigns
    # each func its first-containing set, which makes the ACT engine reload
    # tables (1.3us a pop) between every Ln<->Exp pair.  Steer the chooser to
    # the one shared set for this compile only (ids are positional, so other
    # entries are emptied rather than removed), then restore.
    import concourse.bacc as _bm
    _orig = _bm.get_activation_tables
    _keep = "natural_log_exp_and_others"

    def _one_set(arch):
        full = _orig(arch)
        A = mybir.ActivationFunctionType
        need = {A.Exp, A.Ln, A.Copy, A.Abs, A.Sign, A.Identity, A.MemsetZero}
        if _keep in full and need <= full[_keep]:
            return {k: (v if k == _keep else set()) for k, v in full.items()}
        return full

    _bm.get_activation_tables = _one_set
    try:
        nc.compile()
    finally:
        _bm.get_activation_tables = _orig
    return nc


_CACHE = {}


def _get_nc(nb, nrows, ncores, reps=1):
    key = (nb, nrows, ncores, reps)
    if key not in _CACHE:
        _CACHE[key] = (build_nc(nb, nrows, ncores, reps), host_constants())
    return _CACHE[key]


def run(audio, trace=False):
    """audio (B, C, T) float32 -> (out (B, C, T) float32, results obj)."""
    B, C, T = audio.shape
    assert C == 2
    F = -(-(T + M) // M)
    nb = F // 128
    assert nb * 128 == F, "frame count must be a multiple of 128"
    nrows = F + 1

    nc, consts = _get_nc(nb, nrows, B)

    audio = np.ascontiguousarray(audio, np.float32)
    in_maps = []
    for core in range(B):
        x = np.zeros((2, nrows, M), np.float32)
        flat = x.reshape(2, nrows * M)
        flat[:, M:M + T] = audio[core]
        in_maps.append({"x": x, **consts})

    res = run_bass_kernel_spmd(nc, in_maps, core_ids=list(range(B)),
                               trace=trace)
    out = np.stack([r["out"][:, :T] for r in res.results])
    return out, res


def kernel(audio):
    return run(audio)[0]
